# Initial kernel scaffold
#
"""GATv2 (2-layer, DGL share_weights) on 8 Trainium2 NeuronCores.

Strategy (self-contained; shapes hardcoded for N=50000, E=800000):
 - Nodes partitioned into 8 contiguous dst-slices of 6250; each core owns the
   incoming edges of its slice (segment softmax + scatter-sum stay local).
 - Layer-0 node features feat0 = x @ W0 are computed redundantly on every
   core (cheaper than an all-gather of the table); stored bf16 in HBM.
 - Per-edge src features are fetched with dma_gather (int16 indices; the
   table is split at row 32768 into lo/hi views to fit int16).
 - Edges are sorted by dst and grouped into 128-dst "node blocks"; per block
   a one-hot matrix S (built on-chip via iota compares) turns the segment
   sum into PSUM-accumulated matmuls.  fd (dst features) are reconstructed
   with a one-hot matmul from the block's own 128 rows, fetched as an extra
   gather tile.
 - Softmax: e = attn . leaky_relu(fs+fd) without max-subtraction (scores are
   O(+-8) in this data; exp is safe in fp32) -> out = (S^T (exp(e) * fs)) / s.
 - Layer-1 table feat1 = h @ W1 is computed per-slice and AllGathered.
"""

import numpy as np
import ml_dtypes

import concourse.bass as bass
import concourse.bacc as bacc
import concourse.tile as tile
import concourse.mybir as mybir
from concourse import library_config
from concourse.bass_utils import run_bass_kernel_spmd

F32 = mybir.dt.float32
BF16 = mybir.dt.bfloat16
I16 = mybir.dt.int16

N = 50000
E = 800000
IN_DIM = 256
HID = 64
H0 = 4
NCLS = 64
SLOPE = 0.2
NC = 8
SLICE = N // NC            # 6250
NBLK = 49                  # 128-dst blocks per core (6272)
PADN = NBLK * 128          # 6272
ZPAD = 128                 # zero block prepended to the L0 table
NPAD0 = ZPAD + 50176       # L0 table rows (node n at row n+128), 50304=393*128
NT0 = NPAD0 // 128         # 393 tiles in phase A
NPAD1 = NC * PADN          # 50176 L1 table rows (node n at owner*6272+local)
LO = 32768                 # int16 split point for both tables

def _caps(dst_list, row_list):
    t_lo = t_hi = 0
    for dst, rows in zip(dst_list, row_list):
        for c in range(NC):
            m = (dst >= c * SLICE) & (dst < (c + 1) * SLICE)
            r = rows[m]
            d = dst[m] - c * SLICE
            for b in range(NBLK):
                rb = r[(d >= b * 128) & (d < (b + 1) * 128)]
                nlo = int((rb < LO).sum())
                nhi = int(rb.size - nlo)
                t_lo = max(t_lo, -(-nlo // 128))
                t_hi = max(t_hi, -(-nhi // 128))
    return t_lo, t_hi


def _pack16(idx):
    """[n*16] int16 -> gather layout [128, n] (16-part wrap, replicated x8)."""
    a = idx.reshape(-1, 16).T
    return np.tile(a, (8, 1)).astype(np.int16)


def _prep_layer(src, dst, rows, t_lo, t_hi, fd_rows_percore,
                lo_dummy, hi_dummy):
    """Host metadata for one layer.

    rows: table row of each edge's src.  fd_rows_percore[c][b*128+p]: table
    row of local node p of block b (dummies for padding rows).
    Returns per-core arrays: idx_lo [NBLK,128,(t_lo+1)*8] i16,
    idx_hi [NBLK,128,(t_hi+1)*8], drc [NBLK,128,T] bf16, drr [NBLK,T*128] bf16
    """
    T = t_lo + t_hi
    out = []
    order = np.argsort(dst, kind="stable")
    src_s, dst_s, rows_s = src[order], dst[order], rows[order]
    for c in range(NC):
        lo_edges = np.searchsorted(dst_s, c * SLICE)
        hi_edges = np.searchsorted(dst_s, (c + 1) * SLICE)
        r_c = rows_s[lo_edges:hi_edges]
        d_c = dst_s[lo_edges:hi_edges] - c * SLICE
        idx_lo = np.full((NBLK, (t_lo + 1) * 128), lo_dummy, np.int64)
        idx_hi = np.full((NBLK, (t_hi + 1) * 128), hi_dummy, np.int64)
        drc = np.full((NBLK, T, 128), -1.0, np.float32)
        for b in range(NBLK):
            s0 = np.searchsorted(d_c, b * 128)
            s1 = np.searchsorted(d_c, (b + 1) * 128)
            rb, db = r_c[s0:s1], d_c[s0:s1] - b * 128
            mlo = rb < LO
            rlo, dlo = rb[mlo], db[mlo]
            rhi, dhi = rb[~mlo], db[~mlo]
            assert rlo.size <= t_lo * 128 and rhi.size <= t_hi * 128
            idx_lo[b, :rlo.size] = rlo
            idx_hi[b, :rhi.size] = rhi - LO
            drc[b, :t_lo].reshape(-1)[:dlo.size] = dlo
            drc[b, t_lo:].reshape(-1)[:dhi.size] = dhi
            # fd slot: tile t_lo of lo call / tile t_hi of hi call
            fr = fd_rows_percore[c][b * 128:(b + 1) * 128]
            flo = np.where(fr < LO, fr, lo_dummy)
            fhi = np.where(fr >= LO, fr - LO, hi_dummy)
            idx_lo[b, t_lo * 128:] = flo
            idx_hi[b, t_hi * 128:] = fhi
        out.append({
            "idx_lo": np.stack([_pack16(idx_lo[b]) for b in range(NBLK)]),
            "idx_hi": np.stack([_pack16(idx_hi[b]) for b in range(NBLK)]),
            "drc": drc.transpose(0, 2, 1).astype(ml_dtypes.bfloat16),  # [NBLK,128,T]
            "drr": drc.reshape(NBLK, 1, T * 128).astype(ml_dtypes.bfloat16),
        })
    return out


def build_program(t_lo0, t_hi0, t_lo1, t_hi1, dbg=False, l0_only=False, ncores=NC, reps=1):
    T0 = t_lo0 + t_hi0
    T1 = t_lo1 + t_hi1
    nc = bacc.Bacc("TRN2", target_bir_lowering=False, debug=False,
                   num_devices=ncores, num_swdge_queues=3)

    # ---- external inputs (per-core values supplied via in_maps) ----
    t_xT = nc.dram_tensor("xT", [IN_DIM, NPAD0], BF16, kind="ExternalInput")
    t_W0 = nc.dram_tensor("W0", [IN_DIM, H0 * HID], BF16, kind="ExternalInput")
    t_W1 = nc.dram_tensor("W1", [H0 * HID, NCLS], BF16, kind="ExternalInput")
    t_a0 = nc.dram_tensor("a0bc", [128, H0 * HID], BF16, kind="ExternalInput")
    t_a1 = nc.dram_tensor("a1bc", [128, NCLS], BF16, kind="ExternalInput")
    t_iota_f = nc.dram_tensor("iota_f", [128, 128], BF16, kind="ExternalInput")
    t_iota_p = nc.dram_tensor("iota_p", [128, 512], BF16, kind="ExternalInput")
    t_ones = nc.dram_tensor("ones_row", [1, 128], BF16, kind="ExternalInput")
    t_ident = nc.dram_tensor("ident", [128, 128], BF16, kind="ExternalInput")
    t_identf = nc.dram_tensor("identf", [128, 128], F32, kind="ExternalInput")
    t_il0 = nc.dram_tensor("il0", [NBLK, 128, (t_lo0 + 1) * 8], I16, kind="ExternalInput")
    t_ih0 = nc.dram_tensor("ih0", [NBLK, 128, (t_hi0 + 1) * 8], I16, kind="ExternalInput")
    t_drc0 = nc.dram_tensor("drc0", [NBLK, 128, T0], BF16, kind="ExternalInput")
    t_drr0 = nc.dram_tensor("drr0", [NBLK, 1, T0 * 128], BF16, kind="ExternalInput")
    t_il1 = nc.dram_tensor("il1", [NBLK, 128, (t_lo1 + 1) * 8], I16, kind="ExternalInput")
    t_ih1 = nc.dram_tensor("ih1", [NBLK, 128, (t_hi1 + 1) * 8], I16, kind="ExternalInput")
    t_drc1 = nc.dram_tensor("drc1", [NBLK, 128, T1], BF16, kind="ExternalInput")
    t_drr1 = nc.dram_tensor("drr1", [NBLK, 1, T1 * 128], BF16, kind="ExternalInput")
    t_out = nc.dram_tensor("out", [PADN, NCLS], F32, kind="ExternalOutput")
    if dbg:
        t_dbg_feat0 = nc.dram_tensor("dbg_feat0", [NPAD0, H0 * HID], BF16,
                                     kind="ExternalOutput")
        t_dbg_h = nc.dram_tensor("dbg_h", [PADN, H0 * HID], F32,
                                 kind="ExternalOutput")
        t_dbg_f1 = nc.dram_tensor("dbg_f1", [NPAD1, NCLS], F32,
                                  kind="ExternalOutput")
    if l0_only:
        T0_ = t_lo0 + t_hi0
        t_dbg_S = nc.dram_tensor("dbg_S", [128, T0_ * 128], BF16, kind="ExternalOutput")
        t_dbg_S2 = nc.dram_tensor("dbg_S2", [128, T0_ * 128], BF16, kind="ExternalOutput")
        t_dbg_fd = nc.dram_tensor("dbg_fd", [128, 256], BF16, kind="ExternalOutput")
        t_dbg_fsl = nc.dram_tensor("dbg_fsl", [128, (t_lo0 + 1) * 256], BF16, kind="ExternalOutput")
        t_dbg_eall = nc.dram_tensor("dbg_eall", [128, T0_ * 4], F32, kind="ExternalOutput")
        t_dbg_exw = nc.dram_tensor("dbg_exw", [128, T0_ * 260], BF16, kind="ExternalOutput")

    with tile.TileContext(nc) as tc:
        with tc.tile_pool(name="dram", bufs=1, space="DRAM") as dram, \
             tc.tile_pool(name="const", bufs=1) as cpool:
            feat0 = dram.tile([NPAD0, H0 * HID], BF16)
            h_dram = dram.tile([PADN, H0 * HID], F32)
            f1slice = dram.tile([PADN, NCLS], F32)

            nc.gpsimd.load_library(library_config.mlp)

            W0sb = cpool.tile([128, 2, H0 * HID], BF16)
            nc.sync.dma_start(W0sb[:, 0, :], t_W0[0:128, :])
            nc.sync.dma_start(W0sb[:, 1, :], t_W0[128:256, :])
            W1sb = cpool.tile([128, 2, NCLS], BF16)
            nc.sync.dma_start(W1sb[:, 0, :], t_W1[0:128, :])
            nc.sync.dma_start(W1sb[:, 1, :], t_W1[128:256, :])
            a0bc = cpool.tile([128, H0 * HID], BF16)
            nc.sync.dma_start(a0bc[:], t_a0[:])
            a1bc = cpool.tile([128, NCLS], BF16)
            nc.sync.dma_start(a1bc[:], t_a1[:])
            iota_f = cpool.tile([128, 128], BF16)
            nc.sync.dma_start(iota_f[:], t_iota_f[:])
            iota_p = cpool.tile([128, 512], BF16)
            nc.sync.dma_start(iota_p[:], t_iota_p[:])
            ones_row = cpool.tile([1, 128], BF16)
            nc.sync.dma_start(ones_row[:], t_ones[:])
            ident = cpool.tile([128, 128], BF16)
            nc.sync.dma_start(ident[:], t_ident[:])
            identf = cpool.tile([128, 128], F32)
            nc.sync.dma_start(identf[:], t_identf[:])

            # ---------------- phase A: feat0 table ----------------
            for _rep in range(reps):
              ag_out = dram.tile([NPAD1, NCLS], F32, addr_space="Shared",
                                 tag=f"ag{_rep}")
              with tc.tile_pool(name="pa", bufs=3) as pa, \
                   tc.tile_pool(name="pa_ps", bufs=2, space="PSUM") as pa_ps:
                  for nt in range(NT0):
                      lhs = pa.tile([128, 2, 128], BF16, tag="xT")
                      nc.sync.dma_start(lhs[:, 0, :], t_xT[0:128, nt * 128:(nt + 1) * 128])
                      nc.sync.dma_start(lhs[:, 1, :], t_xT[128:256, nt * 128:(nt + 1) * 128])
                      ps = pa_ps.tile([128, H0 * HID], F32, tag="ps")
                      nc.tensor.matmul(ps[:], lhsT=lhs[:, 0, :], rhs=W0sb[:, 0, :],
                                       start=True, stop=False)
                      nc.tensor.matmul(ps[:], lhsT=lhs[:, 1, :], rhs=W0sb[:, 1, :],
                                       start=False, stop=True)
                      fe = pa.tile([128, H0 * HID], BF16, tag="feat")
                      nc.scalar.copy(fe[:], ps[:])
                      nc.sync.dma_start(feat0[nt * 128:(nt + 1) * 128, :], fe[:])

              # ---------------- phase B: layer-0 edges ----------------
              def edge_layer(layer, T, t_lo, t_hi, tbl, D, H, t_il, t_ih,
                             t_drc, t_drr, abc, out_cb):
                  """D: feat width per head * heads (gather row width).
                  out_cb(b, U_psum, pool): consume the per-block result."""
                  HD = D  # gather row width in elems
                  with tc.tile_pool(name=f"pb{layer}", bufs=3) as pb, \
                       tc.tile_pool(name=f"pb{layer}_e", bufs=2) as pe, \
                       tc.tile_pool(name=f"pb{layer}_ps", bufs=3, space="PSUM") as pps, \
                       tc.tile_pool(name=f"pb{layer}_u", bufs=2, space="PSUM") as pu:
                      for b in range(NBLK):
                          il = pb.tile([128, (t_lo + 1) * 8], I16, tag="il")
                          nc.sync.dma_start(il[:], t_il[b])
                          ih = pb.tile([128, (t_hi + 1) * 8], I16, tag="ih")
                          nc.sync.dma_start(ih[:], t_ih[b])
                          gdt = BF16 if layer == 0 else F32
                          fsl_r = pb.tile([128, t_lo + 1, HD], gdt, tag="fsl")
                          nc.gpsimd.dma_gather(
                              fsl_r[:], tbl[0:LO, :], il[:],
                              (t_lo + 1) * 128, (t_lo + 1) * 128, HD,
                              single_packet=False, queue_num=b % 3)
                          fsh_r = pb.tile([128, t_hi + 1, HD], gdt, tag="fsh")
                          nc.gpsimd.dma_gather(
                              fsh_r[:], tbl[LO:, :], ih[:],
                              (t_hi + 1) * 128, (t_hi + 1) * 128, HD,
                              single_packet=False, queue_num=(b + 1) % 3)
                          if layer == 0:
                              fsl, fsh = fsl_r, fsh_r
                          else:
                              fsl = pb.tile([128, t_lo + 1, HD], BF16, tag="fslb")
                              nc.vector.tensor_copy(fsl[:], fsl_r[:])
                              fsh = pb.tile([128, t_hi + 1, HD], BF16, tag="fshb")
                              nc.vector.tensor_copy(fsh[:], fsh_r[:])
                          fd = pb.tile([128, HD], BF16, tag="fd")
                          nc.vector.tensor_add(fd[:], fsl[:, t_lo, :], fsh[:, t_hi, :])

                          drc = pb.tile([128, T], BF16, tag="drc")
                          nc.sync.dma_start(drc[:], t_drc[b])
                          drr = pb.tile([1, T * 128], BF16, tag="drr")
                          nc.sync.dma_start(drr[:], t_drr[b])
                          S = pb.tile([128, T, 128], BF16, tag="S")
                          nc.vector.tensor_tensor(
                              out=S[:],
                              in0=drc[:].rearrange("p (t o) -> p t o", o=1).to_broadcast([128, T, 128]),
                              in1=iota_f[:].rearrange("p (o n) -> p o n", o=1).to_broadcast([128, T, 128]),
                              op=mybir.AluOpType.is_equal)
                          S2 = pb.tile([128, T * 128], BF16, tag="S2")
                          for g in range((T * 128 + 511) // 512):
                              gw = min(512, T * 128 - g * 512)
                              bc = pps.tile([128, 512], F32, tag="bc")
                              nc.tensor.matmul(bc[:, :gw], lhsT=ones_row[:],
                                               rhs=drr[:, g * 512:g * 512 + gw],
                                               start=True, stop=True)
                              nc.vector.tensor_tensor(
                                  out=S2[:, g * 512:g * 512 + gw],
                                  in0=bc[:, :gw], in1=iota_p[:, :gw],
                                  op=mybir.AluOpType.is_equal)

                          if l0_only and layer == 0 and b == 0:
                              nc.sync.dma_start(t_dbg_S[:], S[:].rearrange("p t n -> p (t n)"))
                              nc.sync.dma_start(t_dbg_S2[:], S2[:])
                              nc.sync.dma_start(t_dbg_fd[:], fd[:])
                              nc.sync.dma_start(t_dbg_fsl[:], fsl[:].rearrange("p t n -> p (t n)"))
                          eall = pe.tile([128, T, H], F32, tag="eall")
                          exw = pe.tile([128, T, H * (HD // H + 1)], BF16, tag="exw")
                          U = pu.tile([128, H * (HD // H + 1)], F32, tag="U")
                          for t in range(T):
                              fs_t = fsl[:, t, :] if t < t_lo else fsh[:, t - t_lo, :]
                              z = pps.tile([128, HD], F32, tag="z")
                              nc.tensor.matmul(z[:], lhsT=S2[:, t * 128:(t + 1) * 128],
                                               rhs=fd[:], start=True, stop=False)
                              nc.tensor.matmul(z[:], lhsT=ident[:], rhs=fs_t,
                                               start=False, stop=True)
                              lr = pb.tile([128, HD], BF16, tag="lr")
                              nc.scalar.activation(lr[:], z[:],
                                                   mybir.ActivationFunctionType.Prelu,
                                                   alpha=SLOPE)
                              am = pb.tile([128, HD], BF16, tag="am")
                              nc.vector.tensor_mul(am[:], lr[:], abc[:])
                              nc.vector.tensor_reduce(
                                  out=eall[:, t, :],
                                  in_=am[:].rearrange("p (h d) -> p h d", h=H),
                                  axis=mybir.AxisListType.X, op=mybir.AluOpType.add)
                          exf = pe.tile([128, T, H], BF16, tag="exf")
                          nc.scalar.activation(exf[:], eall[:],
                                               mybir.ActivationFunctionType.Exp)
                          # ex into exw[:, :, h*(d+1)] slots, wfs into the rest
                          exw4 = exw[:].rearrange("p t (h dd) -> p t h dd", h=H)
                          nc.vector.tensor_copy(exw4[:, :, :, 0:1], exf[:].rearrange("p t (h o) -> p t h o", o=1))
                          for part, fsp, tl in ((0, fsl, t_lo), (1, fsh, t_hi)):
                              t0 = 0 if part == 0 else t_lo
                              nc.vector.tensor_mul(
                                  exw4[:, t0:t0 + tl, :, 1:],
                                  fsp[:, 0:tl, :].rearrange("p t (h d) -> p t h d", h=H),
                                  exf[:, t0:t0 + tl, :].rearrange("p t (h o) -> p t h o", o=1).to_broadcast([128, tl, H, HD // H]))
                          if l0_only and layer == 0 and b == 0:
                              nc.sync.dma_start(t_dbg_eall[:], eall[:].rearrange("p t h -> p (t h)"))
                              nc.sync.dma_start(t_dbg_exw[:], exw[:].rearrange("p t w -> p (t w)"))
                          for t in range(T):
                              nc.tensor.matmul(U[:], lhsT=S[:, t, :],
                                               rhs=exw[:, t, :],
                                               start=(t == 0), stop=(t == T - 1))
                          out_cb(b, U, pb)

              # layer-0 consumer: divide, ELU, store h block
              def l0_out(b, U, pb):
                  U4 = U[:].rearrange("p (h dd) -> p h dd", h=H0)
                  smax = pb.tile([128, H0], F32, tag="smax")
                  nc.vector.tensor_scalar_max(smax[:], U4[:, :, 0], 1e-30)
                  rs = pb.tile([128, H0], F32, tag="rs")
                  nc.vector.reciprocal(rs[:], smax[:])
                  hb = pb.tile([128, H0 * HID], F32, tag="hb")
                  nc.vector.tensor_mul(
                      hb[:].rearrange("p (h d) -> p h d", h=H0),
                      U4[:, :, 1:],
                      rs[:].rearrange("p (h o) -> p h o", o=1).to_broadcast([128, H0, HID]))
                  # ELU = relu(h) + min(exp(min(h,0)) - 1, 0)
                  mn = pb.tile([128, H0 * HID], F32, tag="mn")
                  nc.vector.tensor_scalar_min(mn[:], hb[:], 0.0)
                  ex2 = pb.tile([128, H0 * HID], F32, tag="ex2")
                  nc.scalar.activation(ex2[:], mn[:], mybir.ActivationFunctionType.Exp)
                  em = pb.tile([128, H0 * HID], F32, tag="em")
                  nc.vector.tensor_scalar(out=em[:], in0=ex2[:], scalar1=-1.0,
                                          scalar2=0.0, op0=mybir.AluOpType.add,
                                          op1=mybir.AluOpType.min)
                  rl = pb.tile([128, H0 * HID], F32, tag="rl")
                  nc.scalar.activation(rl[:], hb[:], mybir.ActivationFunctionType.Relu)
                  ho = pb.tile([128, H0 * HID], F32, tag="ho")
                  nc.vector.tensor_add(ho[:], rl[:], em[:])
                  nc.sync.dma_start(h_dram[b * 128:(b + 1) * 128, :], ho[:])

              edge_layer(0, T0, t_lo0, t_hi0, feat0[:], H0 * HID, H0,
                         t_il0, t_ih0, t_drc0, t_drr0, a0bc, l0_out)

              # ---------------- phase C: feat1 slice + AllGather ----------------
              if l0_only:
                  nc.sync.dma_start(t_dbg_h[:], h_dram[:])
              else:
                  with tc.tile_pool(name="pc", bufs=3) as pc, \
                       tc.tile_pool(name="pc_ps", bufs=2, space="PSUM") as pc_ps:
                      for b in range(NBLK):
                          hb = pc.tile([128, H0 * HID], F32, tag="hb")
                          nc.sync.dma_start(hb[:], h_dram[b * 128:(b + 1) * 128, :])
                          f1ps = pc_ps.tile([128, NCLS], F32, tag="f1")
                          for half in range(2):
                              tp = pc_ps.tile([128, 128], F32, tag="tp")
                              nc.tensor.transpose(tp[:], hb[:, half * 128:(half + 1) * 128],
                                                  identf[:])
                              hT = pc.tile([128, 128], BF16, tag="hT")
                              nc.scalar.copy(hT[:], tp[:])
                              nc.tensor.matmul(f1ps[:], lhsT=hT[:], rhs=W1sb[:, half, :],
                                               start=(half == 0), stop=(half == 1))
                          f1 = pc.tile([128, NCLS], F32, tag="f1sb")
                          nc.scalar.copy(f1[:], f1ps[:])
                          nc.sync.dma_start(f1slice[b * 128:(b + 1) * 128, :], f1[:])
                  nc.gpsimd.collective_compute(
                      "AllGather", mybir.AluOpType.bypass,
                      replica_groups=[list(range(NC))],
                      ins=[f1slice[:]], outs=[ag_out[:]])

                  # ---------------- phase D: layer-1 edges ----------------
                  def l1_out(b, U, pb):
                      s1 = pb.tile([128, 1], F32, tag="s1")
                      nc.vector.tensor_scalar_max(s1[:], U[:, 0:1], 1e-30)
                      rs = pb.tile([128, 1], F32, tag="rs1")
                      nc.vector.reciprocal(rs[:], s1[:])
                      ob = pb.tile([128, NCLS], F32, tag="ob")
                      nc.vector.tensor_mul(ob[:], U[:, 1:], rs[:].to_broadcast([128, NCLS]))
                      nc.sync.dma_start(t_out[b * 128:(b + 1) * 128, :], ob[:])

                  edge_layer(1, T1, t_lo1, t_hi1, ag_out[:], NCLS, 1,
                             t_il1, t_ih1, t_drc1, t_drr1, a1bc, l1_out)

            if dbg:
                nc.sync.dma_start(t_dbg_feat0[:], feat0[:])
                if not l0_only:
                    nc.sync.dma_start(t_dbg_h[:], h_dram[:])
                    nc.sync.dma_start(t_dbg_f1[:], ag_out[:])

    nc.compile()
    return nc


def _host_prep(x, W0, attn0, W1, attn1, src0, dst0, src1, dst1):
    bf = ml_dtypes.bfloat16
    src0 = np.asarray(src0); dst0 = np.asarray(dst0)
    src1 = np.asarray(src1); dst1 = np.asarray(dst1)
    rows0 = src0.astype(np.int64) + ZPAD
    owner1 = src1 // SLICE
    rows1 = owner1.astype(np.int64) * PADN + (src1 - owner1 * SLICE)
    t_lo0, t_hi0 = _caps([dst0], [rows0])
    t_lo1, t_hi1 = _caps([dst1], [rows1])

    # per-core fd rows
    fd0 = []
    fd1 = []
    for c in range(NC):
        g = c * SLICE + np.arange(PADN)
        fd0.append(np.where(g < N, g + ZPAD, 0))          # pad -> zero block
        fd1.append(c * PADN + np.arange(PADN))            # pad rows are zero
    meta0 = _prep_layer(src0, dst0, rows0, t_lo0, t_hi0, fd0,
                        lo_dummy=0, hi_dummy=NPAD0 - 1 - LO)
    meta1 = _prep_layer(src1, dst1, rows1, t_lo1, t_hi1, fd1,
                        lo_dummy=SLICE, hi_dummy=7 * PADN + SLICE - LO)

    xT = np.zeros((IN_DIM, NPAD0), dtype=bf)
    xT[:, ZPAD:ZPAD + N] = np.ascontiguousarray(np.asarray(x).T).astype(bf)
    consts = {
        "xT": xT,
        "W0": np.asarray(W0).astype(bf),
        "W1": np.asarray(W1).astype(bf),
        "a0bc": np.tile(np.asarray(attn0).reshape(1, -1), (128, 1)).astype(bf),
        "a1bc": np.tile(np.asarray(attn1).reshape(1, -1), (128, 1)).astype(bf),
        "iota_f": np.tile(np.arange(128, dtype=np.float32)[None, :], (128, 1)).astype(bf),
        "iota_p": np.tile(np.arange(128, dtype=np.float32)[:, None], (1, 512)).astype(bf),
        "ones_row": np.ones((1, 128), dtype=bf),
        "ident": np.eye(128, dtype=np.float32).astype(bf),
        "identf": np.eye(128, dtype=np.float32),
    }
    in_maps = []
    for c in range(NC):
        m = dict(consts)
        m["il0"] = meta0[c]["idx_lo"]
        m["ih0"] = meta0[c]["idx_hi"]
        m["drc0"] = meta0[c]["drc"]
        m["drr0"] = meta0[c]["drr"]
        m["il1"] = meta1[c]["idx_lo"]
        m["ih1"] = meta1[c]["idx_hi"]
        m["drc1"] = meta1[c]["drc"]
        m["drr1"] = meta1[c]["drr"]
        in_maps.append(m)
    return (t_lo0, t_hi0, t_lo1, t_hi1), in_maps


def kernel(x, W0, attn0, W1, attn1, src0, dst0, src1, dst1):
    caps, in_maps = _host_prep(x, W0, attn0, W1, attn1,
                               src0, dst0, src1, dst1)
    nc = build_program(*caps)
    res = run_bass_kernel_spmd(nc, in_maps, core_ids=list(range(NC)))
    out = np.empty((N, NCLS), np.float32)
    for c in range(NC):
        out[c * SLICE:(c + 1) * SLICE] = res.results[c]["out"][:SLICE]
    return out



# revision 32
# speedup vs baseline: 16.0272x; 16.0272x over previous
"""GATv2 (2-layer, DGL share_weights) on 8 Trainium2 NeuronCores.

Strategy (self-contained; shapes hardcoded for N=50000, E=800000):
 - Nodes partitioned into 8 contiguous dst-slices of 6250; each core owns the
   incoming edges of its slice (segment softmax + scatter-sum stay local).
 - Layer-0 node features feat0 = x @ W0 computed redundantly on every core;
   stored bf16 in HBM, node n at row n+128 (row block 0 zeros, tail pad
   rows zero).  Edges sorted by dst, grouped into 128-dst blocks; per block
   one-hot matrices S (edge->dst) / S2 (dst->edge) turn the fd-broadcast and
   the segment-sum into PSUM matmuls.  Per-edge src rows fetched with gpsimd
   dma_gather (int16 idx; tables split into <=32768-row pieces).
 - Per-block-exact tile counts (max over the 8 cores at each block index).
 - S built n-major ([e,n,t]) against a materialized iota constant so the
   DVE 2x mode applies; scores batched per block (Prelu on 2/8-tile PSUM
   groups, multiply-by-attn, then a 2-step add-tree + short tensor_reduce).
 - Layer-1 table: bf16 [feat|feat] duplicated 128-elem rows (256B gather
   rows), held in 3 Shared piece tensors AllGathered early (after L0 blocks
   20/41/48) so the collectives overlap the fused L0->h@W1 block pipeline.
 - Layer 1 gathers fd PER EDGE (extra idx tiles appended to the dst-piece
   gather call) and forms z = fs + fd with a 2x-mode DVE add in SBUF --
   no S2/broadcast matmuls needed for layer 1 at all.
"""

import numpy as np
import ml_dtypes

import concourse.bass as bass
import concourse.bacc as bacc
import concourse.tile as tile
import concourse.mybir as mybir
from concourse import library_config
from concourse.bass_utils import run_bass_kernel_spmd

F32 = mybir.dt.float32
BF16 = mybir.dt.bfloat16
I16 = mybir.dt.int16

N = 50000
E = 800000
IN_DIM = 256
HID = 64
H0 = 4
NCLS = 64
SLOPE = 0.2
NC = 8
SLICE = N // NC            # 6250
NBLK = 49                  # 128-dst blocks per core (6272)
PADN = NBLK * 128          # 6272
ZPAD = 128                 # zero block prepended to the L0 table
NPAD0 = ZPAD + 50176       # L0 table rows (node n at row n+128)
NT0 = NPAD0 // 128         # 393 tiles in phase A
LO = 32768                 # L0 int16 split point

# Layer-1 table: 7 chunks of 896 locals per core, grouped into 3 pieces
# (3/3/1 chunks).  Within a piece rows are core-major then chunk-major.
CBLK = 7                   # blocks per chunk
CROWS = CBLK * 128         # 896
PIECE_CHUNKS = [3, 3, 1]
PIECE_K0 = [0, 3, 6]
PIECE_CROWS = [k * CROWS for k in PIECE_CHUNKS]        # per-core rows
PIECE_ROWS = [NC * r for r in PIECE_CROWS]             # 21504,21504,7168
PIECE_BASE = [0, PIECE_ROWS[0], PIECE_ROWS[0] + PIECE_ROWS[1]]
CHUNKP = [0, 0, 0, 1, 1, 1, 2]                         # chunk -> piece
NPAD1 = NC * PADN          # 50176 total L1 rows


def _row1(c, l):
    """Global L1 table row for core c, local node l (arrays ok)."""
    k = l // CROWS
    P = np.take(CHUNKP, k)
    base = np.take(PIECE_BASE, P)
    pcr = np.take(PIECE_CROWS, P)
    k0 = np.take(PIECE_K0, P)
    return base + c * pcr + (k - k0) * CROWS + (l % CROWS)


def _pack16(idx):
    """[n*16] int -> gather layout [128, n] (16-part wrap, replicated x8)."""
    a = idx.reshape(-1, 16).T
    return np.tile(a, (8, 1)).astype(np.int16)


def _prep_layer(dst, rows, bounds, fd_cfg):
    """Host metadata for one layer with per-block, per-piece tile counts.

    bounds: piece row boundaries [b0, b1, ..., bK] (rows assumed in range).
    fd_cfg: ("split2", fd_rows_percore, dummies) -> every piece gets a +1 fd
            tile; lanes whose fd row is elsewhere point at the piece's zero
            row `dummies[p]` (rel).
            ("perblock", fd_rel_rows_percore, piece_of_block) -> only piece
            piece_of_block[b] gets the +1 fd tile.
    Returns (tcnt[p][b], per-core meta dicts).
    """
    K = len(bounds) - 1
    order = np.argsort(dst, kind="stable")
    dst_s, rows_s = dst[order], rows[order]
    percore = []
    cnt = np.zeros((K, NC, NBLK), np.int64)
    for c in range(NC):
        e0 = np.searchsorted(dst_s, c * SLICE)
        e1 = np.searchsorted(dst_s, (c + 1) * SLICE)
        r_c = rows_s[e0:e1]
        d_c = dst_s[e0:e1] - c * SLICE
        blocks = []
        for b in range(NBLK):
            s0 = np.searchsorted(d_c, b * 128)
            s1 = np.searchsorted(d_c, (b + 1) * 128)
            rb, db = r_c[s0:s1], d_c[s0:s1] - b * 128
            ps = []
            for p in range(K):
                mp = (rb >= bounds[p]) & (rb < bounds[p + 1])
                ps.append((rb[mp] - bounds[p], db[mp]))
                cnt[p, c, b] = int(mp.sum())
            blocks.append(ps)
        percore.append(blocks)
    tcnt = [[int(-(-cnt[p, :, b].max() // 128)) for b in range(NBLK)]
            for p in range(K)]

    mode = fd_cfg[0]
    metas = []
    for c in range(NC):
        meta_cols = []
        drr_cols = []
        for b in range(NBLK):
            T = sum(tcnt[p][b] for p in range(K))
            drc = np.full((T, 128), -1.0, np.float32)
            # per-edge fd rows (piece-rel), tile-order layout; pads -> 0
            if mode == "perblock":
                fdx = np.zeros((T, 128), np.int64)
                frow = fd_cfg[1][c][b * 128:(b + 1) * 128]
            cum = 0
            sec = []
            for p in range(K):
                tp = tcnt[p][b]
                rel, db = percore[c][b][p]
                if mode == "perblock":
                    fdx[cum:cum + tp].reshape(-1)[:db.size] = frow[db]
                fd_here = (mode == "split2") or (fd_cfg[2][b] == p)
                g = tp + ((1 if mode == "split2" else T) if fd_here else 0)
                idx = np.zeros((g * 128,), np.int64)
                idx[:rel.size] = rel
                drc[cum:cum + tp].reshape(-1)[:db.size] = db
                if fd_here:
                    if mode == "split2":
                        fr = fd_cfg[1][c][b * 128:(b + 1) * 128]
                        fr_g = fr - bounds[p]
                        idx[tp * 128:] = np.where(
                            (fr >= bounds[p]) & (fr < bounds[p + 1]),
                            fr_g, fd_cfg[2][p])
                if g:
                    sec.append(_pack16(idx))
                cum += tp
            if mode == "perblock":
                # fd idx tiles appended after piece P's src tiles
                P = fd_cfg[2][b]
                ins = sum(1 for q in range(P + 1) if tcnt[q][b] + (T if fd_cfg[2][b] == q else 0) > 0) - 1
                base = sec[ins]
                tpP = tcnt[P][b]
                base[:, tpP * 8:] = _pack16(fdx.reshape(-1))
            drc_bf = drc.T.astype(ml_dtypes.bfloat16)       # [128, T]
            sec.append(drc_bf.view(np.int16))
            meta_cols.append(np.concatenate(sec, axis=1))
            drr_cols.append(drc.reshape(1, -1).astype(ml_dtypes.bfloat16))
        metas.append({
            "meta": np.concatenate(meta_cols, axis=1),
            "drr": np.concatenate(drr_cols, axis=1),
        })
    return tcnt, metas


def build_program(tcnt0, tcnt1, ncores=NC, reps=1):
    FDP1 = [CHUNKP[b // CBLK] for b in range(NBLK)]
    # gather tile counts (incl fd tiles)
    gcnt0 = [[tcnt0[p][b] + 1 for b in range(NBLK)] for p in range(2)]
    T0 = [sum(tcnt0[p][b] for p in range(2)) for b in range(NBLK)]
    T1 = [sum(tcnt1[p][b] for p in range(3)) for b in range(NBLK)]
    gcnt1 = [[tcnt1[p][b] + (T1[b] if FDP1[b] == p else 0) for b in range(NBLK)]
             for p in range(3)]
    W0cols = [sum(gcnt0[p][b] for p in range(2)) * 8 + T0[b] for b in range(NBLK)]
    W1cols = [sum(gcnt1[p][b] for p in range(3)) * 8 + T1[b] for b in range(NBLK)]

    nc = bacc.Bacc("TRN2", target_bir_lowering=False, debug=False,
                   num_devices=ncores, num_swdge_queues=3)

    t_xT = nc.dram_tensor("xT", [IN_DIM, NPAD0], BF16, kind="ExternalInput")
    t_W0 = nc.dram_tensor("W0", [IN_DIM, H0 * HID], BF16, kind="ExternalInput")
    t_W1 = nc.dram_tensor("W1", [H0 * HID, NCLS], BF16, kind="ExternalInput")
    T0M, T1M = max(T0), max(T1)
    t_a0 = nc.dram_tensor("a0rep", [128, T0M * H0 * HID], BF16, kind="ExternalInput")
    t_a1 = nc.dram_tensor("a1rep", [128, T1M * NCLS], BF16, kind="ExternalInput")
    TM = max(T0M, T1M)
    t_iota_f = nc.dram_tensor("iota_f", [128, 128], BF16, kind="ExternalInput")
    t_iota_nt = nc.dram_tensor("iota_nt", [128, 128 * TM], BF16, kind="ExternalInput")
    t_iota_p = nc.dram_tensor("iota_p", [128, 512], BF16, kind="ExternalInput")
    t_ones = nc.dram_tensor("ones_row", [1, 128], BF16, kind="ExternalInput")
    t_ident = nc.dram_tensor("ident", [128, 128], BF16, kind="ExternalInput")
    t_meta0 = nc.dram_tensor("meta0", [128, sum(W0cols)], I16, kind="ExternalInput")
    t_drr0 = nc.dram_tensor("drr0", [1, sum(T0) * 128], BF16, kind="ExternalInput")
    t_meta1 = nc.dram_tensor("meta1", [128, sum(W1cols)], I16, kind="ExternalInput")
    t_drr1 = nc.dram_tensor("drr1", [1, sum(T1) * 128], BF16, kind="ExternalInput")
    t_out = nc.dram_tensor("out", [PADN, NCLS], F32, kind="ExternalOutput")

    moff0 = np.cumsum([0] + W0cols).tolist()
    doff0 = np.cumsum([0] + [t * 128 for t in T0]).tolist()
    moff1 = np.cumsum([0] + W1cols).tolist()
    doff1 = np.cumsum([0] + [t * 128 for t in T1]).tolist()

    with tile.TileContext(nc) as tc:
        with tc.tile_pool(name="dram", bufs=1, space="DRAM") as dram, \
             tc.tile_pool(name="const", bufs=1) as cpool:
            feat0 = dram.tile([NPAD0, H0 * HID], BF16)
            f1slice = dram.tile([PADN, 2 * NCLS], BF16)

            nc.gpsimd.load_library(library_config.mlp)

            W0sb = cpool.tile([128, 2, H0 * HID], BF16)
            nc.sync.dma_start(W0sb[:, 0, :], t_W0[0:128, :])
            nc.sync.dma_start(W0sb[:, 1, :], t_W0[128:256, :])
            W1sb = cpool.tile([128, 2, NCLS], BF16)
            nc.sync.dma_start(W1sb[:, 0, :], t_W1[0:128, :])
            nc.sync.dma_start(W1sb[:, 1, :], t_W1[128:256, :])
            a0rep = cpool.tile([128, T0M * H0 * HID], BF16)
            nc.sync.dma_start(a0rep[:], t_a0[:])
            a1rep = cpool.tile([128, T1M * NCLS], BF16)
            nc.sync.dma_start(a1rep[:], t_a1[:])
            iota_f = cpool.tile([128, 128], BF16)
            nc.sync.dma_start(iota_f[:], t_iota_f[:])
            iota_nt = cpool.tile([128, 128 * TM], BF16)
            nc.sync.dma_start(iota_nt[:], t_iota_nt[:])
            iota_p = cpool.tile([128, 512], BF16)
            nc.sync.dma_start(iota_p[:], t_iota_p[:])
            ones_row = cpool.tile([1, 128], BF16)
            nc.sync.dma_start(ones_row[:], t_ones[:])
            ident = cpool.tile([128, 128], BF16)
            nc.sync.dma_start(ident[:], t_ident[:])

            for _rep in range(reps):
              ag = [dram.tile([PIECE_ROWS[p], 2 * NCLS], BF16,
                              addr_space="Shared", tag=f"ag{_rep}_{p}",
                              name=f"ag{_rep}_{p}")
                    for p in range(3)]
              # ---------------- phase A: feat0 table ----------------
              with tc.tile_pool(name="pa", bufs=3) as pa, \
                   tc.tile_pool(name="pa_ps", bufs=2, space="PSUM") as pa_ps:
                  AG4 = 4
                  for nt4 in range((NT0 + AG4 - 1) // AG4):
                      n_in = min(AG4, NT0 - AG4 * nt4)
                      ps = pa_ps.tile([128, AG4, H0 * HID], F32, tag="ps")
                      lhs = pa.tile([128, 2, AG4 * 128], BF16, tag="xT")
                      nc.sync.dma_start(
                          lhs[:, :, 0:n_in * 128],
                          t_xT[:, AG4 * nt4 * 128:(AG4 * nt4 + n_in) * 128]
                          .rearrange("(a p) n -> p a n", a=2))
                      for i in range(n_in):
                          nc.tensor.matmul(ps[:, i, :],
                                           lhsT=lhs[:, 0, i * 128:(i + 1) * 128],
                                           rhs=W0sb[:, 0, :],
                                           start=True, stop=False)
                          nc.tensor.matmul(ps[:, i, :],
                                           lhsT=lhs[:, 1, i * 128:(i + 1) * 128],
                                           rhs=W0sb[:, 1, :],
                                           start=False, stop=True)
                      fe = pa.tile([128, AG4, H0 * HID], BF16, tag="feat")
                      nc.scalar.copy(fe[:, 0:n_in, :], ps[:, 0:n_in, :])
                      nc.sync.dma_start(
                          feat0[AG4 * nt4 * 128:(AG4 * nt4 + n_in) * 128, :]
                          .rearrange("(a p) f -> p a f", p=128),
                          fe[:, 0:n_in, :])

              # ---------------- edge phase (shared L0/L1) ----------------
              def edge_layer(layer, tbls, tcnt, gcnt, fd_add, fd_piece,
                             GD, HD, H, t_meta, t_drr, wcols, moff, doff,
                             abc, out_cb):
                  K = len(tbls)
                  D = HD // H
                  gmax = [max(gcnt[p]) for p in range(K)]
                  tmax = max(sum(tcnt[p][b] for p in range(K))
                             for b in range(NBLK))
                  wmax = max(wcols)
                  with tc.tile_pool(name=f"pb{layer}", bufs=3) as pb, \
                       tc.tile_pool(name=f"pb{layer}_e", bufs=3) as pe, \
                       tc.tile_pool(name=f"pb{layer}_z", bufs=2, space="PSUM") as pz, \
                       tc.tile_pool(name=f"pb{layer}_bc", bufs=2, space="PSUM") as pbc, \
                       tc.tile_pool(name=f"pb{layer}_u", bufs=2, space="PSUM") as pu:
                      for b in range(NBLK):
                          tp = [tcnt[p][b] for p in range(K)]
                          gp = [gcnt[p][b] for p in range(K)]
                          cum = np.cumsum([0] + tp).tolist()
                          T = cum[-1]
                          m = pb.tile([128, wmax], I16, tag="meta")
                          nc.scalar.dma_start(
                              m[:, 0:wcols[b]],
                              t_meta[:, moff[b]:moff[b] + wcols[b]])
                          if fd_add:
                              drr = pb.tile([1, tmax * 128], BF16, tag="drr")
                              nc.scalar.dma_start(
                                  drr[:, 0:T * 128],
                                  t_drr[:, doff[b]:doff[b] + T * 128])
                          fs = []
                          woff = 0
                          for p in range(K):
                              fsp = pb.tile([128, gmax[p], GD], BF16,
                                            tag=f"fs{p}")
                              fs.append(fsp)
                              if gp[p]:
                                  nc.gpsimd.dma_gather(
                                      fsp[:, 0:gp[p], :], tbls[p],
                                      m[:, woff:woff + gp[p] * 8],
                                      gp[p] * 128, gp[p] * 128, GD,
                                      single_packet=False,
                                      queue_num=(b + p) % 3)
                              woff += gp[p] * 8
                          if fd_add:
                              fd = pb.tile([128, HD], BF16, tag="fd")
                              nc.vector.tensor_add(fd[:],
                                                   fs[0][:, tp[0], 0:HD],
                                                   fs[1][:, tp[1], 0:HD])
                              fd_ap = fd[:]
                          else:
                              P = fd_piece[b]
                              fd_e = fs[P][:, tp[P]:tp[P] + T, :]
                          drc = m[:, woff:woff + T].bitcast(BF16)
                          S = pb.tile([128, 128, tmax], BF16, tag="S")
                          nc.vector.tensor_tensor(
                              out=S[:, :, 0:T],
                              in0=drc.rearrange("p (o t) -> p o t", o=1)
                                  .to_broadcast([128, 128, T]),
                              in1=iota_nt[:]
                                  .rearrange("p (n t) -> p n t", t=TM)[:, :, 0:T],
                              op=mybir.AluOpType.is_equal)
                          if fd_add:
                              S2 = pb.tile([128, tmax * 128], BF16, tag="S2")
                              for g in range((T * 128 + 511) // 512):
                                  gw = min(512, T * 128 - g * 512)
                                  bc = pbc.tile([128, 512], F32, tag="bc")
                                  nc.tensor.matmul(bc[:, :gw], lhsT=ones_row[:],
                                                   rhs=drr[:, g * 512:g * 512 + gw],
                                                   start=True, stop=True)
                                  nc.vector.tensor_tensor(
                                      out=S2[:, g * 512:g * 512 + gw],
                                      in0=bc[:, :gw], in1=iota_p[:, :gw],
                                      op=mybir.AluOpType.is_equal)

                          def fs_t(t):
                              for p in range(K):
                                  if t < cum[p + 1]:
                                      return fs[p][:, t - cum[p], 0:HD]

                          lr = pb.tile([128, tmax, HD], BF16, tag="lr")
                          if fd_add:
                              ZG = 2
                              for g in range((T + ZG - 1) // ZG):
                                  gn = min(ZG, T - ZG * g)
                                  z = pz.tile([128, ZG, HD], F32, tag="z")
                                  for i in range(gn):
                                      t = ZG * g + i
                                      nc.tensor.matmul(
                                          z[:, i, :],
                                          lhsT=S2[:, t * 128:(t + 1) * 128],
                                          rhs=fd_ap, start=True, stop=False)
                                      nc.tensor.matmul(z[:, i, :], lhsT=ident[:],
                                                       rhs=fs_t(t),
                                                       start=False, stop=True)
                                  nc.scalar.activation(
                                      lr[:, ZG * g:ZG * g + gn, :], z[:, 0:gn, :],
                                      mybir.ActivationFunctionType.Prelu,
                                      alpha=SLOPE)
                          else:
                              zsb = pe.tile([128, tmax, HD], BF16, tag="zsb")
                              for p in range(K):
                                  if tp[p] == 0:
                                      continue
                                  c0 = cum[p]
                                  nc.vector.tensor_add(
                                      zsb[:, c0:c0 + tp[p], :],
                                      fs[p][:, 0:tp[p], 0:HD],
                                      fd_e[:, c0:c0 + tp[p], 0:HD])
                              nc.scalar.activation(
                                  lr[:, 0:T, :], zsb[:, 0:T, :],
                                  mybir.ActivationFunctionType.Prelu,
                                  alpha=SLOPE)
                          am = pe.tile([128, tmax, HD], BF16, tag="am")
                          nc.vector.tensor_mul(
                              am[:, 0:T, :], lr[:, 0:T, :],
                              abc[:, 0:T * HD]
                              .rearrange("p (t hd) -> p t hd", hd=HD))
                          am4 = am[:].rearrange("p t (h d) -> p t h d", h=H)
                          rt1 = pe.tile([128, tmax, H, D // 2], BF16, tag="rt1")
                          nc.vector.tensor_add(rt1[:, 0:T], am4[:, 0:T, :, 0:D // 2],
                                               am4[:, 0:T, :, D // 2:])
                          rt2 = pe.tile([128, tmax, H, D // 4], BF16, tag="rt2")
                          nc.vector.tensor_add(rt2[:, 0:T], rt1[:, 0:T, :, 0:D // 4],
                                               rt1[:, 0:T, :, D // 4:])
                          eall = pe.tile([128, tmax, H], F32, tag="eall")
                          nc.vector.tensor_reduce(
                              out=eall[:, 0:T, :],
                              in_=rt2[:, 0:T],
                              axis=mybir.AxisListType.X, op=mybir.AluOpType.add)
                          # exw row layout: [ex (H cols) | wfs (H*D cols)];
                          # exp writes the ex slots directly (no DVE copy)
                          exw = pe.tile([128, tmax, H + H * D], BF16, tag="exw")
                          exf = exw[:, :, 0:H]
                          nc.scalar.activation(exf[:, 0:T, :], eall[:, 0:T, :],
                                               mybir.ActivationFunctionType.Exp)
                          for p in range(K):
                              if tp[p] == 0:
                                  continue
                              c0 = cum[p]
                              nc.vector.tensor_mul(
                                  exw[:, c0:c0 + tp[p], H:]
                                  .rearrange("p t (h d) -> p t h d", h=H),
                                  fs[p][:, 0:tp[p], 0:HD]
                                  .rearrange("p t (h d) -> p t h d", h=H),
                                  exf[:, c0:c0 + tp[p], :]
                                  .rearrange("p t (h o) -> p t h o", o=1)
                                  .to_broadcast([128, tp[p], H, D]))
                          U = pu.tile([128, H * (D + 1)], F32, tag="U")
                          for t in range(T):
                              nc.tensor.matmul(U[:], lhsT=S[:, :, t],
                                               rhs=exw[:, t, :],
                                               start=(t == 0), stop=(t == T - 1))
                          out_cb(b, U, pb, pe)

              # ---- layer 0 consumer: softmax-div, ELU, fused h@W1, AG ----
              with tc.tile_pool(name="pc_ps", bufs=1, space="PSUM") as pc_ps:
                  def l0_out(b, U, pb, pe):
                      smax = pb.tile([128, H0], F32, tag="smax")
                      nc.vector.tensor_scalar_max(smax[:], U[:, 0:H0], 1e-30)
                      rs = pb.tile([128, H0], F32, tag="rs")
                      nc.vector.reciprocal(rs[:], smax[:])
                      hb = pb.tile([128, H0 * HID], BF16, tag="hb")
                      nc.vector.tensor_mul(
                          hb[:].rearrange("p (h d) -> p h d", h=H0),
                          U[:, H0:].rearrange("p (h d) -> p h d", h=H0),
                          rs[:].rearrange("p (h o) -> p h o", o=1)
                              .to_broadcast([128, H0, HID]))
                      # ELU = relu(h) + min(exp(h) - 1, 0)
                      ex2 = pb.tile([128, H0 * HID], BF16, tag="ex2")
                      nc.scalar.activation(ex2[:], hb[:],
                                           mybir.ActivationFunctionType.Exp)
                      em = pb.tile([128, H0 * HID], BF16, tag="em")
                      nc.vector.tensor_scalar(out=em[:], in0=ex2[:],
                                              scalar1=-1.0, scalar2=0.0,
                                              op0=mybir.AluOpType.add,
                                              op1=mybir.AluOpType.min)
                      rl = pb.tile([128, H0 * HID], BF16, tag="rl")
                      nc.scalar.activation(rl[:], hb[:],
                                           mybir.ActivationFunctionType.Relu)
                      ho = pb.tile([128, H0 * HID], BF16, tag="ho")
                      nc.vector.tensor_add(ho[:], rl[:], em[:])
                      # fused phase C: f1 = (h @ W1) as bf16 [f|f] pair row
                      f1ps = pc_ps.tile([128, NCLS], F32, tag="f1")
                      tpp = pc_ps.tile([128, 2, 128], BF16, tag="tp")
                      for half in range(2):
                          nc.tensor.transpose(
                              tpp[:, half, :], ho[:, half * 128:(half + 1) * 128],
                              ident[:])
                      hT = pb.tile([128, 2, 128], BF16, tag="hT")
                      nc.scalar.copy(hT[:], tpp[:])
                      for half in range(2):
                          nc.tensor.matmul(f1ps[:], lhsT=hT[:, half, :],
                                           rhs=W1sb[:, half, :],
                                           start=(half == 0), stop=(half == 1))
                      f1 = pb.tile([128, 2 * NCLS], BF16, tag="f1sb")
                      nc.scalar.copy(f1[:, 0:NCLS], f1ps[:])
                      nc.vector.tensor_copy(f1[:, NCLS:], f1ps[:])
                      nc.sync.dma_start(f1slice[b * 128:(b + 1) * 128, :], f1[:])
                      for p in range(3):
                          if b == (PIECE_K0[p] + PIECE_CHUNKS[p]) * CBLK - 1:
                              r0 = PIECE_K0[p] * CROWS
                              nc.gpsimd.collective_compute(
                                  "AllGather", mybir.AluOpType.bypass,
                                  replica_groups=[list(range(NC))],
                                  ins=[f1slice[r0:r0 + PIECE_CROWS[p], :]],
                                  outs=[ag[p][:]])

                  edge_layer(0, [feat0[0:LO, :], feat0[LO:, :]], tcnt0, gcnt0,
                             True, None, H0 * HID, H0 * HID, H0,
                             t_meta0, t_drr0, W0cols, moff0, doff0, a0rep,
                             l0_out)

              # ---------------- layer 1 edges ----------------
              def l1_out(b, U, pb, pe):
                  s1 = pb.tile([128, 1], F32, tag="s1")
                  nc.vector.tensor_scalar_max(s1[:], U[:, 0:1], 1e-30)
                  rs = pb.tile([128, 1], F32, tag="rs1")
                  nc.vector.reciprocal(rs[:], s1[:])
                  ob = pb.tile([128, NCLS], F32, tag="ob")
                  nc.vector.tensor_mul(ob[:], U[:, 1:],
                                       rs[:].to_broadcast([128, NCLS]))
                  nc.sync.dma_start(t_out[b * 128:(b + 1) * 128, :], ob[:])

              edge_layer(1, [ag[0][:], ag[1][:], ag[2][:]], tcnt1, gcnt1,
                         False, FDP1, 2 * NCLS, NCLS, 1,
                         t_meta1, t_drr1, W1cols, moff1, doff1, a1rep,
                         l1_out)

    nc.compile()
    return nc


def _host_prep(x, W0, attn0, W1, attn1, src0, dst0, src1, dst1):
    bf = ml_dtypes.bfloat16
    src0 = np.asarray(src0); dst0 = np.asarray(dst0)
    src1 = np.asarray(src1); dst1 = np.asarray(dst1)
    rows0 = src0.astype(np.int64) + ZPAD
    owner1 = (src1 // SLICE).astype(np.int64)
    rows1 = _row1(owner1, (src1 - owner1 * SLICE).astype(np.int64))

    fd0 = []
    fd1 = []
    for c in range(NC):
        g = c * SLICE + np.arange(PADN)
        fd0.append(np.where(g < N, g + ZPAD, 0))
        loc = np.arange(PADN)
        r = _row1(c, loc)
        P = np.take(CHUNKP, loc // CROWS)
        fd1.append(r - np.take(PIECE_BASE, P))   # piece-relative
    tcnt0, meta0 = _prep_layer(
        dst0, rows0, [0, LO, NPAD0],
        ("split2", fd0, [0, NPAD0 - 1 - LO]))
    FDP1 = [CHUNKP[b // CBLK] for b in range(NBLK)]
    tcnt1, meta1 = _prep_layer(
        dst1, rows1, [0, PIECE_BASE[1], PIECE_BASE[2], NPAD1],
        ("perblock", fd1, FDP1))

    T0M = max(sum(tcnt0[p][b] for p in range(2)) for b in range(NBLK))
    T1M = max(sum(tcnt1[p][b] for p in range(3)) for b in range(NBLK))
    xT = np.zeros((IN_DIM, NPAD0), dtype=bf)
    xT[:, ZPAD:ZPAD + N] = np.ascontiguousarray(np.asarray(x).T).astype(bf)
    consts = {
        "xT": xT,
        "W0": np.asarray(W0).astype(bf),
        "W1": np.asarray(W1).astype(bf),
        "a0rep": np.tile(np.asarray(attn0).reshape(1, -1), (128, T0M)).astype(bf),
        "a1rep": np.tile(np.asarray(attn1).reshape(1, -1), (128, T1M)).astype(bf),
        "iota_f": np.tile(np.arange(128, dtype=np.float32)[None, :], (128, 1)).astype(bf),
        "iota_nt": np.tile(np.repeat(np.arange(128, dtype=np.float32),
                                     max(T0M, T1M))[None, :], (128, 1)).astype(bf),
        "iota_p": np.tile(np.arange(128, dtype=np.float32)[:, None], (1, 512)).astype(bf),
        "ones_row": np.ones((1, 128), dtype=bf),
        "ident": np.eye(128, dtype=np.float32).astype(bf),
    }
    in_maps = []
    for c in range(NC):
        mm = dict(consts)
        mm["meta0"] = meta0[c]["meta"]
        mm["drr0"] = meta0[c]["drr"]
        mm["meta1"] = meta1[c]["meta"]
        mm["drr1"] = meta1[c]["drr"]
        in_maps.append(mm)
    return (tcnt0, tcnt1), in_maps


def kernel(x, W0, attn0, W1, attn1, src0, dst0, src1, dst1):
    caps, in_maps = _host_prep(x, W0, attn0, W1, attn1,
                               src0, dst0, src1, dst1)
    nc = build_program(*caps)
    res = run_bass_kernel_spmd(nc, in_maps, core_ids=list(range(NC)))
    out = np.empty((N, NCLS), np.float32)
    for c in range(NC):
        out[c * SLICE:(c + 1) * SLICE] = res.results[c]["out"][:SLICE]
    return out
